# revision 1
# baseline (speedup 1.0000x reference)
"""MoE 2D router kernel for 8 Trainium2 NeuronCores.

Strategy (pure data parallel, batch-sharded):
  - B=16 batches split across 8 cores (2 per core). Per core, each batch's
    [C=16, H=128, W=128] tensor is viewed as [128, 2048] in SBUF with
    partition p = c*8 + blk (blk = pixel-block of 2048 contiguous pixels),
    so channel params are per-partition scalars and HBM loads are fully
    contiguous.
  - Expert-axis (C) reductions (top-2 max) are done by PE-transposing
    Hlogits chunks to pixel-major layout (PE f32 transpose is bit-exact),
    then free-axis strided tensor_reduce; per-pixel m1/m2 are broadcast back
    to (c, pixel) layout with 0/1 selection matmuls on the PE (bit-exact),
    so the argmax mask is an exact is_equal directly in (c, pixel) layout.
    The softmax denominator + its channel broadcast is a single PE matmul
    against a block-diagonal ones matrix.
  - softplus(t) = Ln(1 + e^t) with the Exp output Newton-refined through the
    Ln table (kills the ~1e-5 Exp-table error before it amplifies through
    the m1/m2 -> erf path); erf(q) = 2*(DGelu(sqrt2 q) - 1.12838*q*e^(-q^2)) - 1
    from the Derivative_Gelu table; softmax computed without max subtraction
    (|Hlogits| < 30 for this distribution).
  - Work is split across DVE (vector), Pool (gpsimd) and ACT (scalar)
    engines; erf tails of both batches run together to avoid activation
    table reloads.
"""
import sys

sys.path.insert(0, "/opt/trn_rl_repo")

import numpy as np

B, C, H, W = 16, 16, 128, 128
NCORES = 8
BPC = B // NCORES           # batches per core
HW = H * W                  # 16384 pixels per (batch, channel)
NBLK = 8                    # pixel blocks per batch (HW / 2048)
FB = C * HW // 128          # free size per batch in [128, FB] layout = 2048
NCH = 4                     # 128-col groups per chunk
CHW = 512                   # chunk width
VB = 4                      # virtual pipeline batches per core
FBV = BPC * FB // VB        # free size per virtual batch = 1024
NCHV = FBV // CHW           # chunks per virtual batch = 2

_CACHE = {}


def _build():
    import concourse.bacc as bacc
    import concourse.mybir as mybir
    from concourse.tile import TileContext

    f32 = mybir.dt.float32
    bf16 = mybir.dt.bfloat16
    AX = mybir.AxisListType
    OP = mybir.AluOpType
    AF = mybir.ActivationFunctionType
    SQRT2 = 1.4142135623730951
    C_ERF = 1.1283791670955126  # 2/sqrt(pi)
    BIGNEG = -1e30

    nc = bacc.Bacc(trn_type="TRN2", target_bir_lowering=False, debug=False,
                   num_devices=NCORES, name="moe_router")

    xd = nc.dram_tensor("x", [BPC, 128, FB], f32, kind="ExternalInput")
    nd = nc.dram_tensor("noise", [BPC, 128, FB], f32, kind="ExternalInput")
    wgp_d = nc.dram_tensor("wgp", [128, 1], f32, kind="ExternalInput")
    wnp_d = nc.dram_tensor("wnp", [128, 1], f32, kind="ExternalInput")
    id_f = nc.dram_tensor("id_f", [128, 128], f32, kind="ExternalInput")
    sel32_d = nc.dram_tensor("sel32", [32, 512], f32, kind="ExternalInput")
    selsum_d = nc.dram_tensor("selsum", [128, 128], f32, kind="ExternalInput")
    gd = nc.dram_tensor("g_out", [BPC, 128, FB], f32, kind="ExternalOutput")
    ld = nc.dram_tensor("load_out", [BPC, 128, FB], f32, kind="ExternalOutput")

    with TileContext(nc) as tc:
        with tc.tile_pool(name="const", bufs=1) as cpool, \
             tc.tile_pool(name="io", bufs=2) as iop, \
             tc.tile_pool(name="work", bufs=2) as wp, \
             tc.tile_pool(name="erf", bufs=1) as ep, \
             tc.tile_pool(name="chunk", bufs=3) as chp, \
             tc.tile_pool(name="ps_t", bufs=2, space="PSUM") as ps_t, \
             tc.tile_pool(name="ps_m", bufs=1, space="PSUM") as ps_m, \
             tc.tile_pool(name="ps_s", bufs=1, space="PSUM") as ps_s, \
             tc.tile_pool(name="ps_b", bufs=1, space="PSUM") as ps_b, \
             tc.tile_pool(name="ps_b2", bufs=2, space="PSUM") as ps_b2:

            qts, wts = [], []
            eu0_is, lc_is, wn_is, et_is = [], [], [], []
            consts_loaded = [None]

            def _load_consts():
                wgp = cpool.tile([128, 1], f32, tag="wgp")
                nc.sync.dma_start(out=wgp[:, :], in_=wgp_d[:, :])
                wnp = cpool.tile([128, 1], f32, tag="wnp")
                nc.sync.dma_start(out=wnp[:, :], in_=wnp_d[:, :])
                idf = cpool.tile([128, 128], f32, tag="idf")
                nc.sync.dma_start(out=idf[:, :], in_=id_f[:, :])
                sel32 = cpool.tile([32, 512], f32, tag="sel32")
                nc.sync.dma_start(out=sel32[:, :], in_=sel32_d[:, :])
                selsum = cpool.tile([128, 128], f32, tag="selsum")
                nc.sync.dma_start(out=selsum[:, :], in_=selsum_d[:, :])
                return wgp, wnp, idf, sel32, selsum

            for b in range(VB):
                bb, bo = divmod(b, VB // BPC)
                bs = bo * FBV
                # ---- load (inputs first so compute starts ASAP; consts are
                # not needed until the transpose stage) ----
                xt = iop.tile([128, FBV], f32, tag="x")
                nc.sync.dma_start(out=xt[:, :], in_=xd[bb, :, bs:bs + FBV])
                nt = iop.tile([128, FBV], f32, tag="noise")
                nc.sync.dma_start(out=nt[:, :], in_=nd[bb, :, bs:bs + FBV])
                if consts_loaded[0] is None:
                    consts_loaded[0] = _load_consts()
                wgp, wnp, idf, sel32, selsum = consts_loaded[0]

                # ---- gates (A-space) ----
                # softplus(t) = Ln(1 + e^t); e^t Newton-refined via the Ln
                # table (y' = y*(1 + t - ln(y))).
                tv = wp.tile([128, FBV], f32, tag="tv")
                nc.vector.tensor_scalar_mul(tv[:, :], xt[:, :], wnp[:, :])
                eu0 = wp.tile([128, FBV], f32, tag="eu0")
                eu0_i = nc.scalar.activation(eu0[:, :], xt[:, :], AF.Exp,
                                             scale=wnp[:, :])
                eu0_is.append(eu0_i)
                lc = wp.tile([128, FBV], f32, tag="lc")
                lc_i = nc.scalar.activation(lc[:, :], eu0[:, :], AF.Ln)
                lc_is.append(lc_i)
                d2 = wp.tile([128, FBV], f32, tag="d2")
                nc.gpsimd.tensor_tensor(d2[:, :], tv[:, :], lc[:, :], op=OP.subtract)
                eu = wp.tile([128, FBV], f32, tag="eu2")
                nc.vector.scalar_tensor_tensor(eu[:, :], d2[:, :], 1.0, eu0[:, :],
                                               op0=OP.add, op1=OP.mult)
                wnoise = wp.tile([128, FBV], f32, tag="wnoise")
                wn_i = nc.scalar.activation(wnoise[:, :], eu[:, :], AF.Ln, bias=1.0)
                wn_is.append(wn_i)
                rw = wp.tile([128, FBV], f32, tag="rw")
                nc.vector.reciprocal(rw[:, :], wnoise[:, :])
                nw = wp.tile([128, FBV], f32, tag="nw")
                nc.gpsimd.tensor_tensor(nw[:, :], nt[:, :], wnoise[:, :], op=OP.mult)
                wg = wp.tile([128, FBV], f32, tag="wg")
                nc.vector.tensor_scalar_mul(wg[:, :], xt[:, :], wgp[:, :])
                hl = wp.tile([128, FBV], f32, tag="hl")
                nc.vector.tensor_tensor(hl[:, :], wg[:, :], nw[:, :], op=OP.add)
                et = wp.tile([128, FBV], f32, tag="e")
                et_i = nc.scalar.activation(et[:, :], hl[:, :], AF.Exp)
                et_is.append(et_i)

                # ---- streamed chunks: reduce, mask, m2, broadcasts, n1/mm ----
                # All m1/m2 movement is exact: PE f32 transpose and 0/1
                # selection matmuls are bit-preserving (HW-verified), so the
                # argmax mask is an exact is_equal in A-space.
                mask_sb = wp.tile([128, FBV], bf16, tag="eu2")
                srecip = wp.tile([128, FBV], f32, tag="d2")
                n1 = wp.tile([128, FBV], f32, tag="eu0")
                mm = wp.tile([128, FBV], f32, tag="lc")
                for ch in range(NCHV):
                    cs = ch * CHW
                    hlT = ps_t.tile([128, CHW], f32, tag="tT")
                    for g in range(NCH):
                        nc.tensor.transpose(
                            hlT[:, g * 128:(g + 1) * 128],
                            hl[:, cs + g * 128:cs + (g + 1) * 128], idf[:, :])
                    vT = hlT[:, :].rearrange("p (g c k) -> p g k c", g=NCH, c=C)
                    m1c = chp.tile([128, 32], f32, tag="m1c")
                    nc.vector.tensor_reduce(m1c[:, :], vT, axis=AX.X, op=OP.max)
                    m1cT_p = ps_b.tile([32, 128], f32, tag="m1cT_p")
                    nc.tensor.transpose(m1cT_p[:, :], m1c[:, :], idf[:, :])
                    m1cT = chp.tile([32, 128], f32, tag="m1cT")
                    nc.vector.tensor_copy(m1cT[:, :], m1cT_p[:, :])
                    m1bA = ps_b2.tile([128, CHW], f32, tag="m1bA")
                    for g in range(NCH):
                        nc.tensor.matmul(m1bA[:, g * 128:(g + 1) * 128],
                                         sel32[:, g * 128:(g + 1) * 128],
                                         m1cT[:, :])
                    # exact argmax mask + loss numerator part 1, in A-space
                    nc.vector.tensor_tensor(mask_sb[:, cs:cs + CHW],
                                            hl[:, cs:cs + CHW],
                                            m1bA[:, :], op=OP.is_equal)
                    nc.vector.tensor_tensor(n1[:, cs:cs + CHW], wg[:, cs:cs + CHW],
                                            m1bA[:, :], op=OP.subtract)
                    # 2nd max directly in T-space: mask+remove the argmax with
                    # a stride-0 broadcast of m1c, no PE round-trip on this path
                    m1b = (m1c[:, :].rearrange("p (g k) -> p g k", g=NCH)
                           .unsqueeze(2).broadcast_to([128, NCH, C, NBLK]))
                    mkT = chp.tile([128, CHW], bf16, tag="mkT")
                    nc.vector.tensor_tensor(mkT[:, :], hlT[:, :], m1b,
                                            op=OP.is_equal)
                    mdT = chp.tile([128, CHW], f32, tag="mdT")
                    nc.vector.scalar_tensor_tensor(
                        mdT[:, :], mkT[:, :], BIGNEG, hlT[:, :],
                        op0=OP.mult, op1=OP.add)
                    vM = mdT[:, :].rearrange("p (g c k) -> p g k c", g=NCH, c=C)
                    m2c = chp.tile([128, 32], f32, tag="m2c")
                    nc.vector.tensor_reduce(m2c[:, :], vM, axis=AX.X, op=OP.max)
                    m2pc = chp.tile([128, 32], f32, tag="m2pc")
                    nc.vector.tensor_tensor(m2pc[:, :], m2c[:, :], m1c[:, :],
                                            op=OP.subtract)
                    m2cT_p = ps_m.tile([32, 128], f32, tag="m2cT_p")
                    nc.tensor.transpose(m2cT_p[:, :], m2pc[:, :], idf[:, :])
                    m2cT = chp.tile([32, 128], f32, tag="m2cT")
                    nc.vector.tensor_copy(m2cT[:, :], m2cT_p[:, :])
                    m2bA = ps_b.tile([128, CHW], f32, tag="m2bA")
                    for g in range(NCH):
                        nc.tensor.matmul(m2bA[:, g * 128:(g + 1) * 128],
                                         sel32[:, g * 128:(g + 1) * 128],
                                         m2cT[:, :])
                    nc.vector.tensor_tensor(mm[:, cs:cs + CHW],
                                            mask_sb[:, cs:cs + CHW],
                                            m2bA[:, :], op=OP.mult)
                    # softmax denominator (+ broadcast over c) on PE
                    ssum = ps_s.tile([128, CHW], f32, tag="ssum")
                    nc.tensor.matmul(ssum[:, :], selsum[:, :], et[:, cs:cs + CHW])
                    nc.vector.reciprocal(srecip[:, cs:cs + CHW], ssum[:, :])

                # ---- G output ----
                g0 = wp.tile([128, FBV], f32, tag="tv")
                nc.gpsimd.tensor_tensor(g0[:, :], mask_sb[:, :], srecip[:, :],
                                        op=OP.mult)
                gt = iop.tile([128, FBV], f32, tag="g")
                nc.gpsimd.tensor_tensor(gt[:, :], g0[:, :], et[:, :], op=OP.mult)
                nc.sync.dma_start(out=gd[bb, :, bs:bs + FBV], in_=gt[:, :])

                # ---- erf argument ----
                numer = wp.tile([128, FBV], f32, tag="nw")
                nc.gpsimd.tensor_tensor(numer[:, :], n1[:, :], mm[:, :], op=OP.subtract)
                qt = ep.tile([128, FBV], f32, tag=f"q{b}")
                nc.gpsimd.tensor_tensor(qt[:, :], numer[:, :], rw[:, :], op=OP.mult)
                z2 = wp.tile([128, FBV], f32, tag="wg")
                nc.gpsimd.tensor_tensor(z2[:, :], qt[:, :], qt[:, :], op=OP.mult)
                wt = ep.tile([128, FBV], f32, tag=f"w{b}")
                wt_inst = nc.scalar.activation(wt[:, :], z2[:, :], AF.Exp, scale=-1.0)
                last_a_inst = wt_inst
                qts.append(qt)
                wts.append(wt)

            # ---- erf tails for all vbatches (one DGelu table load) ----
            from concourse.tile import add_dep_helper
            for b in range(VB):
                bb, bo = divmod(b, VB // BPC)
                bs = bo * FBV
                qt, wt = qts[b], wts[b]
                dg = wp.tile([128, FBV], f32, tag="hl")
                dg_inst = nc.scalar.activation(dg[:, :], qt[:, :],
                                               AF.Derivative_Gelu, scale=SQRT2)
                add_dep_helper(last_a_inst.ins, dg_inst.ins, sync=True,
                               reason="group DGelu after all Exp/Ln act ops")
                # erf tail, refactored so t2 is a plain Pool tensor_tensor:
                # load = C*((2/C)*dg - q*w) - 1  ==  2*dg - C*q*w - 1
                t2 = wp.tile([128, FBV], f32, tag="eu0")
                nc.gpsimd.tensor_tensor(t2[:, :], qt[:, :], wt[:, :], op=OP.mult)
                er = wp.tile([128, FBV], f32, tag="lc")
                nc.vector.scalar_tensor_tensor(er[:, :], dg[:, :], 2.0 / C_ERF,
                                               t2[:, :], op0=OP.mult,
                                               op1=OP.subtract)
                lt = iop.tile([128, FBV], f32, tag="load")
                nc.vector.tensor_scalar(lt[:, :], er[:, :], C_ERF, 1.0,
                                        op0=OP.mult, op1=OP.subtract)
                nc.sync.dma_start(out=ld[bb, :, bs:bs + FBV], in_=lt[:, :])

    nc.compile()
    # NOTE: retargeting Exp/Ln table loads to the combined
    # natural_log_exp_and_others table saves ~8 table loads (~10us ACT) but
    # that table's entries are numerically coarser: load_loss absmax degrades
    # from 5e-4 to 6e-3 through the softplus->m1/m2->erf path. Keep the
    # per-function tables.
    return nc


def _fix_act_tables(nc, mybir):
    """Retarget Exp/Ln activation-table loads to a single table containing
    both functions, then drop loads that reload the already-active table.
    The default placement assigns per-function tables, producing a 1.3us
    table load at nearly every Exp<->Ln transition."""
    from concourse.hw_specs import get_activation_tables
    AFT = mybir.ActivationFunctionType
    tabs = list(get_activation_tables(nc.m.arch).items())
    union_id = None
    for i, (_, fs) in enumerate(tabs):
        if AFT.Exp in fs and AFT.Ln in fs:
            union_id = i
            break
    assert union_id is not None
    union_funcs = tabs[union_id][1]
    for blk in nc.m.functions[0].blocks:
        insts = blk.instructions
        # retarget each load according to the activations it serves
        loads = []
        for idx, inst in enumerate(insts):
            if isinstance(inst, mybir.InstLoadActFuncSet):
                loads.append((idx, inst))
        for li, (idx, load) in enumerate(loads):
            end = loads[li + 1][0] if li + 1 < len(loads) else len(insts)
            funcs = {i2.func for i2 in insts[idx + 1:end]
                     if isinstance(i2, mybir.InstActivation)}
            if funcs and funcs.issubset(union_funcs):
                load.act_func_set_id = union_id
        # drop redundant consecutive loads (keep any that carry sem waits)
        cur = None
        to_remove = []
        for inst in insts:
            if isinstance(inst, mybir.InstLoadActFuncSet):
                if inst.act_func_set_id == cur and not inst.has_wait():
                    to_remove.append(inst)
                else:
                    cur = inst.act_func_set_id
            elif isinstance(inst, mybir.InstActivation):
                assert inst.func in tabs[cur][1], (inst.func, cur)
        for inst in to_remove:
            insts.remove(inst)


def _consts():
    identity = np.eye(128, dtype=np.float32)
    # sel32[j*8 + blk, j*128 + c*8 + blk] = 1 : broadcast row (j,blk) of the
    # chunk-local [32,128] m-rows over the 16 channels of group j.
    sel32 = np.zeros((32, 512), dtype=np.float32)
    for j in range(4):
        for blk in range(8):
            for c in range(C):
                sel32[j * 8 + blk, j * 128 + c * 8 + blk] = 1.0
    selsum = np.zeros((128, 128), dtype=np.float32)
    for cp in range(C):
        for blk in range(8):
            for c in range(C):
                selsum[cp * 8 + blk, c * 8 + blk] = 1.0
    return {
        "id_f": identity,
        "sel32": sel32,
        "selsum": selsum,
    }


def make_in_maps(x, noise, wg_param, wnoise_param):
    consts = _consts()
    wgp = np.repeat(np.ascontiguousarray(wg_param, dtype=np.float32).reshape(C), 8
                    ).reshape(128, 1)
    wnp = np.repeat(np.ascontiguousarray(wnoise_param, dtype=np.float32).reshape(C), 8
                    ).reshape(128, 1)
    x = np.ascontiguousarray(x, dtype=np.float32)
    noise = np.ascontiguousarray(noise, dtype=np.float32)
    in_maps = []
    for i in range(NCORES):
        xs = x[i * BPC:(i + 1) * BPC].reshape(BPC, 128, FB)
        ns = noise[i * BPC:(i + 1) * BPC].reshape(BPC, 128, FB)
        in_maps.append({"x": xs, "noise": ns, "wgp": wgp, "wnp": wnp, **consts})
    return in_maps


def kernel(x, noise, wg_param, wnoise_param):
    from concourse.bass_utils import run_bass_kernel_spmd

    if "nc" not in _CACHE:
        _CACHE["nc"] = _build()
    nc = _CACHE["nc"]
    in_maps = make_in_maps(x, noise, wg_param, wnoise_param)
    res = run_bass_kernel_spmd(nc, in_maps, list(range(NCORES)))
    G = np.empty((B, C, H, W), dtype=np.float32)
    L = np.empty((B, C, H, W), dtype=np.float32)
    for i in range(NCORES):
        G[i * BPC:(i + 1) * BPC] = res.results[i]["g_out"].reshape(BPC, C, H, W)
        L[i * BPC:(i + 1) * BPC] = res.results[i]["load_out"].reshape(BPC, C, H, W)
    return G, L



# revision 15
# speedup vs baseline: 81.6253x; 81.6253x over previous
"""MoE 2D router kernel for 8 Trainium2 NeuronCores.

Strategy (pure data parallel, batch-sharded):
  - B=16 batches split across 8 cores (2 per core). Per core, each batch's
    [C=16, H=128, W=128] tensor is viewed as [128, 2048] in SBUF with
    partition p = c*8 + blk (blk = pixel-block of 2048 contiguous pixels),
    so channel params are per-partition scalars and HBM loads are fully
    contiguous.
  - Expert-axis (C) reductions (top-2 max) are done by PE-transposing
    Hlogits chunks to pixel-major layout (PE f32 transpose is bit-exact),
    then free-axis strided tensor_reduce; per-pixel m1/m2 are broadcast back
    to (c, pixel) layout with 0/1 selection matmuls on the PE (bit-exact),
    so the argmax mask is an exact is_equal directly in (c, pixel) layout.
    The softmax denominator + its channel broadcast is a single PE matmul
    against a block-diagonal ones matrix.
  - softplus(t) = Ln(1 + e^t) with the Exp output Newton-refined through the
    Ln table (kills the ~1e-5 Exp-table error before it amplifies through
    the m1/m2 -> erf path); erf(q) = 2*(DGelu(sqrt2 q) - 1.12838*q*e^(-q^2)) - 1
    from the Derivative_Gelu table; softmax computed without max subtraction
    (|Hlogits| < 30 for this distribution).
  - Work is split across DVE (vector), Pool (gpsimd) and ACT (scalar)
    engines; erf tails of both batches run together to avoid activation
    table reloads.
"""
import sys

sys.path.insert(0, "/opt/trn_rl_repo")

import numpy as np

B, C, H, W = 16, 16, 128, 128
NCORES = 8
BPC = B // NCORES           # batches per core
HW = H * W                  # 16384 pixels per (batch, channel)
NBLK = 8                    # pixel blocks per batch (HW / 2048)
FB = C * HW // 128          # free size per batch in [128, FB] layout = 2048
NCH = 4                     # 128-col groups per chunk
CHW = 512                   # chunk width
VB = 4                      # virtual pipeline batches per core
FBV = BPC * FB // VB        # free size per virtual batch = 1024
NCHV = FBV // CHW           # chunks per virtual batch = 2

_CACHE = {}


def _build(reps=1):
    """Build the router NEFF. reps>1 wraps the whole per-call body in a
    hardware For_i loop that recomputes the same outputs `reps` times —
    used only for timing (per-execute tunnel overhead through axon is
    ~5-6ms, ~50x the device time, so device time is measured as
    wall/reps on a repeated body)."""
    import concourse.bacc as bacc
    import concourse.mybir as mybir
    from concourse.tile import TileContext
    import contextlib

    f32 = mybir.dt.float32
    bf16 = mybir.dt.bfloat16
    AX = mybir.AxisListType
    OP = mybir.AluOpType
    AF = mybir.ActivationFunctionType
    SQRT2 = 1.4142135623730951
    C_ERF = 1.1283791670955126  # 2/sqrt(pi)
    BIGNEG = -1e30

    nc = bacc.Bacc(trn_type="TRN2", target_bir_lowering=False, debug=False,
                   num_devices=NCORES, name="moe_router",
                   enable_partition_id=False)

    # Single merged input/output: per-call runtime operands through the axon
    # PJRT path cost ~1.5-2ms EACH (measured), so everything rides in one
    # ExternalInput and one ExternalOutput. Layout of in_d planes:
    #   [0:BPC]       x batches      [128, FB]
    #   [BPC:2*BPC]   noise batches  [128, FB]
    #   [2*BPC]       params plane: col 0 = wgp (128), col 1 = wnp (128)
    # out_d planes: [0:BPC] = G, [BPC:2*BPC] = load_loss.
    in_d = nc.dram_tensor("in_all", [2 * BPC + 1, 128, FB], f32,
                          kind="ExternalInput")
    out_d = nc.dram_tensor("out_all", [2 * BPC, 128, FB], f32,
                           kind="ExternalOutput")
    cdata = _consts()
    id_f = nc.inline_tensor(cdata["id_f"], name="id_f")
    sel32_d = nc.inline_tensor(cdata["sel32"], name="sel32")
    selsum_d = nc.inline_tensor(cdata["selsum"], name="selsum")

    with TileContext(nc) as tc:
        with tc.tile_pool(name="const", bufs=1) as cpool, \
             tc.tile_pool(name="io", bufs=2) as iop, \
             tc.tile_pool(name="work", bufs=2) as wp, \
             tc.tile_pool(name="erf", bufs=1) as ep, \
             tc.tile_pool(name="chunk", bufs=3) as chp, \
             tc.tile_pool(name="ps_t", bufs=2, space="PSUM") as ps_t, \
             tc.tile_pool(name="ps_m", bufs=1, space="PSUM") as ps_m, \
             tc.tile_pool(name="ps_s", bufs=1, space="PSUM") as ps_s, \
             tc.tile_pool(name="ps_b", bufs=1, space="PSUM") as ps_b, \
             tc.tile_pool(name="ps_b2", bufs=2, space="PSUM") as ps_b2:

            qts, wts = [], []
            eu0_is, lc_is, wn_is, et_is = [], [], [], []
            consts_loaded = [None]

            def _load_consts():
                wgp = cpool.tile([128, 1], f32, tag="wgp")
                nc.sync.dma_start(out=wgp[:, :], in_=in_d[2 * BPC, :, 0:1])
                wnp = cpool.tile([128, 1], f32, tag="wnp")
                nc.sync.dma_start(out=wnp[:, :], in_=in_d[2 * BPC, :, 1:2])
                idf = cpool.tile([128, 128], f32, tag="idf")
                nc.sync.dma_start(out=idf[:, :], in_=id_f[:, :])
                sel32 = cpool.tile([32, 512], f32, tag="sel32")
                nc.sync.dma_start(out=sel32[:, :], in_=sel32_d[:, :])
                selsum = cpool.tile([128, 128], f32, tag="selsum")
                nc.sync.dma_start(out=selsum[:, :], in_=selsum_d[:, :])
                return wgp, wnp, idf, sel32, selsum

            _loop = contextlib.ExitStack()
            if reps > 1:
                _loop.enter_context(tc.For_i(0, reps, 1))

            for b in range(VB):
                bb, bo = divmod(b, VB // BPC)
                bs = bo * FBV
                # ---- load (inputs first so compute starts ASAP; consts are
                # not needed until the transpose stage) ----
                xt = iop.tile([128, FBV], f32, tag="x")
                nc.sync.dma_start(out=xt[:, :], in_=in_d[bb, :, bs:bs + FBV])
                nt = iop.tile([128, FBV], f32, tag="noise")
                nc.sync.dma_start(out=nt[:, :], in_=in_d[BPC + bb, :, bs:bs + FBV])
                if consts_loaded[0] is None:
                    consts_loaded[0] = _load_consts()
                wgp, wnp, idf, sel32, selsum = consts_loaded[0]

                # ---- gates (A-space) ----
                # softplus(t) = Ln(1 + e^t); table error ~1e-3 through the
                # m1/m2 -> erf path is fine for the 2e-2 gate.
                eu0 = wp.tile([128, FBV], f32, tag="eu0")
                eu0_i = nc.scalar.activation(eu0[:, :], xt[:, :], AF.Exp,
                                             scale=wnp[:, :])
                eu0_is.append(eu0_i)
                wnoise = wp.tile([128, FBV], f32, tag="wnoise")
                wn_i = nc.scalar.activation(wnoise[:, :], eu0[:, :], AF.Ln, bias=1.0)
                wn_is.append(wn_i)
                rw = wp.tile([128, FBV], f32, tag="rw")
                nc.vector.reciprocal_approx_fast(rw[:, :], wnoise[:, :])
                nw = wp.tile([128, FBV], f32, tag="nw")
                nc.gpsimd.tensor_tensor(nw[:, :], nt[:, :], wnoise[:, :], op=OP.mult)
                wg = wp.tile([128, FBV], f32, tag="wg")
                nc.vector.tensor_scalar_mul(wg[:, :], xt[:, :], wgp[:, :])
                hl = wp.tile([128, FBV], f32, tag="hl")
                nc.vector.tensor_tensor(hl[:, :], wg[:, :], nw[:, :], op=OP.add)
                et = wp.tile([128, FBV], f32, tag="e")
                et_i = nc.scalar.activation(et[:, :], hl[:, :], AF.Exp)
                et_is.append(et_i)

                # ---- streamed chunks: reduce, mask, m2, broadcasts, n1/mm ----
                # All m1/m2 movement is exact: PE f32 transpose and 0/1
                # selection matmuls are bit-preserving (HW-verified), so the
                # argmax mask is an exact is_equal in A-space.
                mask_sb = wp.tile([128, FBV], bf16, tag="eu2")
                srecip = wp.tile([128, FBV], f32, tag="d2")
                n1 = wp.tile([128, FBV], f32, tag="eu0")
                mm = wp.tile([128, FBV], f32, tag="lc")
                for ch in range(NCHV):
                    cs = ch * CHW
                    hlT = ps_t.tile([128, CHW], f32, tag="tT")
                    for g in range(NCH):
                        nc.tensor.transpose(
                            hlT[:, g * 128:(g + 1) * 128],
                            hl[:, cs + g * 128:cs + (g + 1) * 128], idf[:, :])
                    vT = hlT[:, :].rearrange("p (g c k) -> p g k c", g=NCH, c=C)
                    m1c = chp.tile([128, 32], f32, tag="m1c")
                    nc.vector.tensor_reduce(m1c[:, :], vT, axis=AX.X, op=OP.max)
                    m1cT_p = ps_b.tile([32, 128], f32, tag="m1cT_p")
                    nc.tensor.transpose(m1cT_p[:, :], m1c[:, :], idf[:, :])
                    m1cT = chp.tile([32, 128], f32, tag="m1cT")
                    nc.vector.tensor_copy(m1cT[:, :], m1cT_p[:, :])
                    m1bA = ps_b2.tile([128, CHW], f32, tag="m1bA")
                    for g in range(NCH):
                        nc.tensor.matmul(m1bA[:, g * 128:(g + 1) * 128],
                                         sel32[:, g * 128:(g + 1) * 128],
                                         m1cT[:, :])
                    # exact argmax mask + loss numerator part 1, in A-space
                    nc.vector.tensor_tensor(mask_sb[:, cs:cs + CHW],
                                            hl[:, cs:cs + CHW],
                                            m1bA[:, :], op=OP.is_equal)
                    nc.vector.tensor_tensor(n1[:, cs:cs + CHW], wg[:, cs:cs + CHW],
                                            m1bA[:, :], op=OP.subtract)
                    # 2nd max directly in T-space: mask+remove the argmax with
                    # a stride-0 broadcast of m1c, no PE round-trip on this path
                    m1b = (m1c[:, :].rearrange("p (g k) -> p g k", g=NCH)
                           .unsqueeze(2).broadcast_to([128, NCH, C, NBLK]))
                    mkT = chp.tile([128, CHW], bf16, tag="mkT")
                    nc.vector.tensor_tensor(mkT[:, :], hlT[:, :], m1b,
                                            op=OP.is_equal)
                    mdT = chp.tile([128, CHW], f32, tag="mdT")
                    nc.vector.scalar_tensor_tensor(
                        mdT[:, :], mkT[:, :], BIGNEG, hlT[:, :],
                        op0=OP.mult, op1=OP.add)
                    vM = mdT[:, :].rearrange("p (g c k) -> p g k c", g=NCH, c=C)
                    m2c = chp.tile([128, 32], f32, tag="m2c")
                    nc.vector.tensor_reduce(m2c[:, :], vM, axis=AX.X, op=OP.max)
                    m2pc = chp.tile([128, 32], f32, tag="m2pc")
                    nc.vector.tensor_tensor(m2pc[:, :], m2c[:, :], m1c[:, :],
                                            op=OP.subtract)
                    m2cT_p = ps_m.tile([32, 128], f32, tag="m2cT_p")
                    nc.tensor.transpose(m2cT_p[:, :], m2pc[:, :], idf[:, :])
                    m2cT = chp.tile([32, 128], f32, tag="m2cT")
                    nc.vector.tensor_copy(m2cT[:, :], m2cT_p[:, :])
                    m2bA = ps_b.tile([128, CHW], f32, tag="m2bA")
                    for g in range(NCH):
                        nc.tensor.matmul(m2bA[:, g * 128:(g + 1) * 128],
                                         sel32[:, g * 128:(g + 1) * 128],
                                         m2cT[:, :])
                    nc.vector.tensor_tensor(mm[:, cs:cs + CHW],
                                            mask_sb[:, cs:cs + CHW],
                                            m2bA[:, :], op=OP.mult)
                    # softmax denominator (+ broadcast over c) on PE
                    ssum = ps_s.tile([128, CHW], f32, tag="ssum")
                    nc.tensor.matmul(ssum[:, :], selsum[:, :], et[:, cs:cs + CHW])
                    nc.vector.reciprocal_approx_fast(srecip[:, cs:cs + CHW],
                                                     ssum[:, :])

                # ---- G output ----
                g0 = wp.tile([128, FBV], f32, tag="tv")
                nc.gpsimd.tensor_tensor(g0[:, :], mask_sb[:, :], srecip[:, :],
                                        op=OP.mult)
                gt = iop.tile([128, FBV], f32, tag="g")
                nc.gpsimd.tensor_tensor(gt[:, :], g0[:, :], et[:, :], op=OP.mult)
                nc.sync.dma_start(out=out_d[bb, :, bs:bs + FBV], in_=gt[:, :])

                # ---- erf argument ----
                numer = wp.tile([128, FBV], f32, tag="nw")
                nc.gpsimd.tensor_tensor(numer[:, :], n1[:, :], mm[:, :], op=OP.subtract)
                qt = ep.tile([128, FBV], f32, tag=f"q{b}")
                nc.gpsimd.tensor_tensor(qt[:, :], numer[:, :], rw[:, :], op=OP.mult)
                last_a_inst = et_i
                qts.append(qt)

            # ---- erf tails for all vbatches (one Erf table load) ----
            from concourse.tile import add_dep_helper
            for b in range(VB):
                bb, bo = divmod(b, VB // BPC)
                bs = bo * FBV
                qt = qts[b]
                lt = iop.tile([128, FBV], f32, tag="load")
                lt_inst = nc.scalar.activation(lt[:, :], qt[:, :], AF.Erf)
                add_dep_helper(last_a_inst.ins, lt_inst.ins, sync=True,
                               reason="group Erf after all Exp/Ln act ops")
                nc.sync.dma_start(out=out_d[BPC + bb, :, bs:bs + FBV], in_=lt[:, :])

            _loop.close()

    nc.compile()
    # Retarget Exp/Ln loads to the combined natural_log_exp_and_others table
    # (coarser entries degrade load_loss absmax ~5e-4 -> ~6e-3, still well
    # under the 2e-2 gate) and drop redundant reloads; Erf keeps its own
    # table, loaded once for the grouped tails.
    _fix_act_tables(nc, mybir)
    return nc


def _fix_act_tables(nc, mybir):
    """Retarget Exp/Ln activation-table loads to a single table containing
    both functions, then drop loads that reload the already-active table.
    The default placement assigns per-function tables, producing a 1.3us
    table load at nearly every Exp<->Ln transition."""
    from concourse.hw_specs import get_activation_tables
    AFT = mybir.ActivationFunctionType
    tabs = list(get_activation_tables(nc.m.arch).items())
    union_id = None
    for i, (_, fs) in enumerate(tabs):
        if AFT.Exp in fs and AFT.Ln in fs:
            union_id = i
            break
    assert union_id is not None
    union_funcs = tabs[union_id][1]
    for blk in nc.m.functions[0].blocks:
        insts = blk.instructions
        # retarget each load according to the activations it serves
        loads = []
        for idx, inst in enumerate(insts):
            if isinstance(inst, mybir.InstLoadActFuncSet):
                loads.append((idx, inst))
        for li, (idx, load) in enumerate(loads):
            end = loads[li + 1][0] if li + 1 < len(loads) else len(insts)
            funcs = {i2.func for i2 in insts[idx + 1:end]
                     if isinstance(i2, mybir.InstActivation)}
            if funcs and funcs.issubset(union_funcs):
                load.act_func_set_id = union_id
        # drop redundant consecutive loads (keep any that carry sem waits)
        cur = None
        to_remove = []
        for inst in insts:
            if isinstance(inst, mybir.InstLoadActFuncSet):
                if inst.act_func_set_id == cur and not inst.has_wait():
                    to_remove.append(inst)
                else:
                    cur = inst.act_func_set_id
            elif isinstance(inst, mybir.InstActivation):
                assert inst.func in tabs[cur][1], (inst.func, cur)
        for inst in to_remove:
            insts.remove(inst)


def _consts():
    identity = np.eye(128, dtype=np.float32)
    # sel32[j*8 + blk, j*128 + c*8 + blk] = 1 : broadcast row (j,blk) of the
    # chunk-local [32,128] m-rows over the 16 channels of group j.
    sel32 = np.zeros((32, 512), dtype=np.float32)
    for j in range(4):
        for blk in range(8):
            for c in range(C):
                sel32[j * 8 + blk, j * 128 + c * 8 + blk] = 1.0
    selsum = np.zeros((128, 128), dtype=np.float32)
    for cp in range(C):
        for blk in range(8):
            for c in range(C):
                selsum[cp * 8 + blk, c * 8 + blk] = 1.0
    return {
        "id_f": identity,
        "sel32": sel32,
        "selsum": selsum,
    }


def make_in_maps(x, noise, wg_param, wnoise_param):
    wgp = np.repeat(np.ascontiguousarray(wg_param, dtype=np.float32).reshape(C), 8)
    wnp = np.repeat(np.ascontiguousarray(wnoise_param, dtype=np.float32).reshape(C), 8)
    x = np.ascontiguousarray(x, dtype=np.float32)
    noise = np.ascontiguousarray(noise, dtype=np.float32)
    in_maps = []
    for i in range(NCORES):
        arr = np.zeros((2 * BPC + 1, 128, FB), dtype=np.float32)
        arr[0:BPC] = x[i * BPC:(i + 1) * BPC].reshape(BPC, 128, FB)
        arr[BPC:2 * BPC] = noise[i * BPC:(i + 1) * BPC].reshape(BPC, 128, FB)
        arr[2 * BPC, :, 0] = wgp
        arr[2 * BPC, :, 1] = wnp
        in_maps.append({"in_all": arr})
    return in_maps


def kernel(x, noise, wg_param, wnoise_param):
    from concourse.bass_utils import run_bass_kernel_spmd

    if "nc" not in _CACHE:
        _CACHE["nc"] = _build()
    nc = _CACHE["nc"]
    in_maps = make_in_maps(x, noise, wg_param, wnoise_param)
    res = run_bass_kernel_spmd(nc, in_maps, list(range(NCORES)))
    G = np.empty((B, C, H, W), dtype=np.float32)
    L = np.empty((B, C, H, W), dtype=np.float32)
    for i in range(NCORES):
        out = res.results[i]["out_all"]
        G[i * BPC:(i + 1) * BPC] = out[0:BPC].reshape(BPC, C, H, W)
        L[i * BPC:(i + 1) * BPC] = out[BPC:2 * BPC].reshape(BPC, C, H, W)
    return G, L



# revision 24
# speedup vs baseline: 111.9030x; 1.3709x over previous
"""MoE 2D router kernel for 8 Trainium2 NeuronCores.

Strategy (pure data parallel, batch-sharded):
  - B=16 batches split across 8 cores (2 per core). Per core, each batch's
    [C=16, H=128, W=128] tensor is viewed as [128, 2048] in SBUF with
    partition p = c*8 + blk (blk = pixel-block of 2048 contiguous pixels),
    so channel params are per-partition scalars and HBM loads are fully
    contiguous.
  - Expert-axis (C) reductions (top-2 max) are done by PE-transposing
    Hlogits chunks to pixel-major layout (PE f32 transpose is bit-exact),
    then free-axis strided tensor_reduce; per-pixel m1/m2 are broadcast back
    to (c, pixel) layout with 0/1 selection matmuls on the PE (bit-exact),
    so the argmax mask is an exact is_equal directly in (c, pixel) layout.
    The softmax denominator + its channel broadcast is a single PE matmul
    against a block-diagonal ones matrix.
  - All runtime data rides in ONE ExternalInput and ONE ExternalOutput
    (per-operand dispatch through the axon PJRT path costs ~1.5-2ms); the
    selection matrices are baked into the NEFF as Const tensors.
  - Emission is stage-ordered across the 4 virtual batches so each
    activation table (exp / ln / erf) loads once per pass, and erf() comes
    from the HW Erf table directly.
  - Reciprocals use the ~51-ULP approx (BITWISE_NOT seed + 2 NR passes),
    5x faster than the bit-exact 6-cycle-per-element divide.
"""
import sys

sys.path.insert(0, "/opt/trn_rl_repo")

import numpy as np

B, C, H, W = 16, 16, 128, 128
NCORES = 8
BPC = B // NCORES           # batches per core
HW = H * W                  # 16384 pixels per (batch, channel)
NBLK = 8                    # pixel blocks per batch (HW / 2048)
FB = C * HW // 128          # free size per batch in [128, FB] layout = 2048
NCH = 4                     # 128-col groups per chunk
CHW = 512                   # chunk width
VB = 4                      # virtual pipeline batches per core
FBV = BPC * FB // VB        # free size per virtual batch = 1024
NCHV = FBV // CHW           # chunks per virtual batch = 2

_CACHE = {}


def _build(reps=1):
    """Build the router NEFF. reps>1 wraps the whole per-call body in a
    hardware For_i loop that recomputes the same outputs `reps` times —
    used only for timing (per-execute tunnel overhead through axon is
    ~5-6ms, ~40x the device time, so device time is measured as
    wall/reps on a repeated body)."""
    import concourse.bacc as bacc
    import concourse.mybir as mybir
    from concourse.tile import TileContext, add_dep_helper
    import contextlib

    f32 = mybir.dt.float32
    bf16 = mybir.dt.bfloat16
    AX = mybir.AxisListType
    OP = mybir.AluOpType
    AF = mybir.ActivationFunctionType
    BIGNEG = -1e30

    nc = bacc.Bacc(trn_type="TRN2", target_bir_lowering=False, debug=False,
                   num_devices=NCORES, name="moe_router",
                   enable_partition_id=False)

    # Plane layout of in_d: [x0, n0, x1, n1, params]; params plane col 0 =
    # wgp (128), col 1 = wnp (128). out_d planes: [g0, l0, g1, l1].
    in_d = nc.dram_tensor("in_all", [2 * BPC + 1, 128, FB], f32,
                          kind="ExternalInput")
    out_d = nc.dram_tensor("out_all", [2 * BPC, 128, FB], f32,
                           kind="ExternalOutput")
    cdata = _consts()
    id_f = nc.inline_tensor(cdata["id_f"], name="id_f")
    sel32_d = nc.inline_tensor(cdata["sel32"], name="sel32")
    selsum_d = nc.inline_tensor(cdata["selsum"], name="selsum")

    with TileContext(nc) as tc:
        with tc.tile_pool(name="const", bufs=1) as cpool, \
             tc.tile_pool(name="io", bufs=2) as iop, \
             tc.tile_pool(name="work", bufs=1) as wp, \
             tc.tile_pool(name="chunk", bufs=2) as chp, \
             tc.tile_pool(name="ps_t", bufs=1, space="PSUM") as ps_t, \
             tc.tile_pool(name="ps_a", bufs=1, space="PSUM") as ps_a, \
             tc.tile_pool(name="ps_b", bufs=1, space="PSUM") as ps_b, \
             tc.tile_pool(name="ps_s", bufs=1, space="PSUM") as ps_s, \
             tc.tile_pool(name="ps_sm", bufs=1, space="PSUM") as ps_sm:

            consts_loaded = [None]

            def _load_consts():
                wgp = cpool.tile([128, 1], f32, tag="wgp")
                nc.sync.dma_start(out=wgp[:, :], in_=in_d[2 * BPC, :, 0:1])
                wnp = cpool.tile([128, 1], f32, tag="wnp")
                nc.sync.dma_start(out=wnp[:, :], in_=in_d[2 * BPC, :, 1:2])
                idf = cpool.tile([128, 128], f32, tag="idf")
                nc.sync.dma_start(out=idf[:, :], in_=id_f[:, :])
                sel32 = cpool.tile([32, 512], f32, tag="sel32")
                nc.sync.dma_start(out=sel32[:, :], in_=sel32_d[:, :])
                selsum = cpool.tile([128, 128], f32, tag="selsum")
                nc.sync.dma_start(out=selsum[:, :], in_=selsum_d[:, :])
                return wgp, wnp, idf, sel32, selsum

            consts_loaded[0] = _load_consts()

            _loop = contextlib.ExitStack()
            if reps > 1:
                _loop.enter_context(tc.For_i(0, reps, 1))

            XT, EU0, HL, RW, QT = ({} for _ in range(5))
            eu0_is, wn_is, et_is = [], [], []

            # ---- stage 1: x loads, eu0 = Exp(x*wnp) [exp table] ----
            for b in range(VB):
                bb, bo = divmod(b, VB // BPC)
                bs = bo * FBV
                xt = iop.tile([128, FBV], f32, tag=f"x{b}")
                nc.sync.dma_start(out=xt[:, :], in_=in_d[2 * bb, :, bs:bs + FBV])
                if consts_loaded[0] is None:
                    consts_loaded[0] = _load_consts()
                wgp, wnp, idf, sel32, selsum = consts_loaded[0]
                XT[b] = xt
                eu0 = wp.tile([128, FBV], f32, tag=f"eu0{b}")
                eu0_is.append(nc.scalar.activation(eu0[:, :], xt[:, :], AF.Exp,
                                                   scale=wnp[:, :]))
                EU0[b] = eu0

            # ---- stage 2: wnoise = Ln(1+eu0) [ln table], nw, hl, rw ----
            for b in range(VB):
                bb, bo = divmod(b, VB // BPC)
                bs = bo * FBV
                nt = iop.tile([128, FBV], f32, tag="nt", bufs=4)
                nc.sync.dma_start(out=nt[:, :],
                                  in_=in_d[2 * bb + 1, :, bs:bs + FBV])
                wn = wp.tile([128, FBV], f32, tag="wn", bufs=2)
                wn_i = nc.scalar.activation(wn[:, :], EU0[b][:, :], AF.Ln,
                                            bias=1.0)
                if b == 0:
                    add_dep_helper(wn_i.ins, eu0_is[VB - 1].ins, sync=True,
                                   reason="group Ln after all Exp (1 table load)")
                wn_is.append(wn_i)
                nw = wp.tile([128, FBV], f32, tag="nw", bufs=2)
                nc.gpsimd.tensor_tensor(nw[:, :], nt[:, :], wn[:, :], op=OP.mult)
                hl = wp.tile([128, FBV], f32, tag=f"hl{b}")
                nc.vector.scalar_tensor_tensor(hl[:, :], XT[b][:, :], wgp[:, :],
                                               nw[:, :], op0=OP.mult, op1=OP.add)
                HL[b] = hl
                rw = wp.tile([128, FBV], f32, tag=f"rw{b}")
                nc.vector.reciprocal_approx_fast(rw[:, :], wn[:, :])
                RW[b] = rw

            # ---- stage 4: et [exp table], reductions, masks, G output ----
            for b in range(VB):
                bb, bo = divmod(b, VB // BPC)
                bs = bo * FBV
                hl, rw = HL[b], RW[b]
                et = wp.tile([128, FBV], f32, tag="et", bufs=2)
                et_i = nc.scalar.activation(et[:, :], hl[:, :], AF.Exp)
                if b == 0:
                    add_dep_helper(et_i.ins, wn_is[VB - 1].ins, sync=True,
                                   reason="group et-Exp after all Ln")
                et_is.append(et_i)
                m1bA = ps_a.tile([128, FBV], f32, tag="m1bA")
                m2bA = ps_b.tile([128, FBV], f32, tag="m2bA")
                srecip = wp.tile([128, FBV], f32, tag="sr", bufs=1)
                for ch in range(NCHV):
                    cs = ch * CHW
                    hlT = ps_t.tile([128, CHW], f32, tag="hlT")
                    for g in range(NCH):
                        nc.tensor.transpose(
                            hlT[:, g * 128:(g + 1) * 128],
                            hl[:, cs + g * 128:cs + (g + 1) * 128], idf[:, :])
                    vT = hlT[:, :].rearrange("p (g c k) -> p g k c", g=NCH, c=C)
                    m1c = chp.tile([128, 32], f32, tag="m1c")
                    nc.vector.tensor_reduce(m1c[:, :], vT, axis=AX.X, op=OP.max)
                    m1cT_p = ps_sm.tile([32, 128], f32, tag="m1p")
                    nc.tensor.transpose(m1cT_p[:, :], m1c[:, :], idf[:, :])
                    m1cT = chp.tile([32, 128], f32, tag="m1s")
                    nc.scalar.copy(m1cT[:, :], m1cT_p[:, :])
                    for g in range(NCH):
                        nc.tensor.matmul(m1bA[:, cs + g * 128:cs + (g + 1) * 128],
                                         sel32[:, g * 128:(g + 1) * 128],
                                         m1cT[:, :])
                    # 2nd max in T-space: mask+remove the argmax with a
                    # stride-0 broadcast of m1c (no PE round-trip here)
                    m1b = (m1c[:, :].rearrange("p (g k) -> p g k", g=NCH)
                           .unsqueeze(2).broadcast_to([128, NCH, C, NBLK]))
                    mkT = chp.tile([128, CHW], bf16, tag="mkT")
                    nc.vector.tensor_tensor(mkT[:, :], hlT[:, :], m1b,
                                            op=OP.is_equal)
                    mdT = chp.tile([128, CHW], f32, tag="mdT")
                    nc.vector.scalar_tensor_tensor(
                        mdT[:, :], mkT[:, :], BIGNEG, hlT[:, :],
                        op0=OP.mult, op1=OP.add)
                    vM = mdT[:, :].rearrange("p (g c k) -> p g k c", g=NCH, c=C)
                    m2c = chp.tile([128, 32], f32, tag="m2c")
                    nc.vector.tensor_reduce(m2c[:, :], vM, axis=AX.X, op=OP.max)
                    m2pc = chp.tile([128, 32], f32, tag="m2pc")
                    nc.vector.tensor_tensor(m2pc[:, :], m2c[:, :], m1c[:, :],
                                            op=OP.subtract)
                    m2cT_p = ps_sm.tile([32, 128], f32, tag="m2p")
                    nc.tensor.transpose(m2cT_p[:, :], m2pc[:, :], idf[:, :])
                    m2cT = chp.tile([32, 128], f32, tag="m2s")
                    nc.scalar.copy(m2cT[:, :], m2cT_p[:, :])
                    for g in range(NCH):
                        nc.tensor.matmul(m2bA[:, cs + g * 128:cs + (g + 1) * 128],
                                         sel32[:, g * 128:(g + 1) * 128],
                                         m2cT[:, :])
                    # softmax denominator (+ broadcast over c) on PE
                    ssum = ps_s.tile([128, CHW], f32, tag="ssum")
                    nc.tensor.matmul(ssum[:, :], selsum[:, :], et[:, cs:cs + CHW])
                    nc.vector.reciprocal_approx_fast(srecip[:, cs:cs + CHW],
                                                     ssum[:, :])

                # exact argmax mask + erf-numerator pieces, full width
                mask = wp.tile([128, FBV], bf16, tag="mask", bufs=2)
                nc.vector.tensor_tensor(mask[:, :], hl[:, :], m1bA[:, :],
                                        op=OP.is_equal)
                n1 = wp.tile([128, FBV], f32, tag="n1", bufs=1)
                nc.vector.scalar_tensor_tensor(n1[:, :], XT[b][:, :], wgp[:, :],
                                               m1bA[:, :], op0=OP.mult,
                                               op1=OP.subtract)
                mm = wp.tile([128, FBV], f32, tag="mm", bufs=1)
                nc.vector.tensor_tensor(mm[:, :], mask[:, :], m2bA[:, :],
                                        op=OP.mult)
                numer = wp.tile([128, FBV], f32, tag="numer", bufs=1)
                nc.gpsimd.tensor_tensor(numer[:, :], n1[:, :], mm[:, :],
                                        op=OP.subtract)
                qt = wp.tile([128, FBV], f32, tag=f"q{b}")
                nc.gpsimd.tensor_tensor(qt[:, :], numer[:, :], rw[:, :],
                                        op=OP.mult)
                QT[b] = qt
                g0 = wp.tile([128, FBV], f32, tag="g0", bufs=1)
                nc.gpsimd.tensor_tensor(g0[:, :], mask[:, :], srecip[:, :],
                                        op=OP.mult)
                gt = iop.tile([128, FBV], f32, tag="g")
                nc.gpsimd.tensor_tensor(gt[:, :], g0[:, :], et[:, :], op=OP.mult)
                nc.sync.dma_start(out=out_d[2 * bb, :, bs:bs + FBV], in_=gt[:, :])

            # ---- stage 5: erf tails (one Erf table load) ----
            for b in range(VB):
                bb, bo = divmod(b, VB // BPC)
                bs = bo * FBV
                lt = iop.tile([128, FBV], f32, tag="load")
                lt_inst = nc.scalar.activation(lt[:, :], QT[b][:, :], AF.Erf)
                if b == 0:
                    add_dep_helper(lt_inst.ins, et_is[VB - 1].ins, sync=True,
                                   reason="group Erf after all Exp")
                nc.sync.dma_start(out=out_d[2 * bb + 1, :, bs:bs + FBV],
                                  in_=lt[:, :])

            _loop.close()

    nc.compile()
    _fix_act_tables(nc, mybir)
    return nc


def _fix_act_tables(nc, mybir):
    """Drop activation-table loads that reload the already-active table
    (keep any that carry semaphore waits)."""
    from concourse.hw_specs import get_activation_tables
    tabs = list(get_activation_tables(nc.m.arch).items())
    for blk in nc.m.functions[0].blocks:
        insts = blk.instructions
        cur = None
        to_remove = []
        for inst in insts:
            if isinstance(inst, mybir.InstLoadActFuncSet):
                if inst.act_func_set_id == cur and not inst.has_wait():
                    to_remove.append(inst)
                else:
                    cur = inst.act_func_set_id
            elif isinstance(inst, mybir.InstActivation):
                assert inst.func in tabs[cur][1], (inst.func, cur)
        for inst in to_remove:
            insts.remove(inst)


def _consts():
    identity = np.eye(128, dtype=np.float32)
    # sel32[j*8 + blk, j*128 + c*8 + blk] = 1 : broadcast row (j,blk) of the
    # chunk-local [32,128] m-rows over the 16 channels of group j.
    sel32 = np.zeros((32, 512), dtype=np.float32)
    for j in range(4):
        for blk in range(8):
            for c in range(C):
                sel32[j * 8 + blk, j * 128 + c * 8 + blk] = 1.0
    selsum = np.zeros((128, 128), dtype=np.float32)
    for cp in range(C):
        for blk in range(8):
            for c in range(C):
                selsum[cp * 8 + blk, c * 8 + blk] = 1.0
    return {
        "id_f": identity,
        "sel32": sel32,
        "selsum": selsum,
    }


def make_in_maps(x, noise, wg_param, wnoise_param):
    wgp = np.repeat(np.ascontiguousarray(wg_param, dtype=np.float32).reshape(C), 8)
    wnp = np.repeat(np.ascontiguousarray(wnoise_param, dtype=np.float32).reshape(C), 8)
    x = np.ascontiguousarray(x, dtype=np.float32)
    noise = np.ascontiguousarray(noise, dtype=np.float32)
    in_maps = []
    for i in range(NCORES):
        arr = np.zeros((2 * BPC + 1, 128, FB), dtype=np.float32)
        for bb in range(BPC):
            arr[2 * bb] = x[i * BPC + bb].reshape(128, FB)
            arr[2 * bb + 1] = noise[i * BPC + bb].reshape(128, FB)
        arr[2 * BPC, :, 0] = wgp
        arr[2 * BPC, :, 1] = wnp
        in_maps.append({"in_all": arr})
    return in_maps


def kernel(x, noise, wg_param, wnoise_param):
    from concourse.bass_utils import run_bass_kernel_spmd

    if "nc" not in _CACHE:
        _CACHE["nc"] = _build()
    nc = _CACHE["nc"]
    in_maps = make_in_maps(x, noise, wg_param, wnoise_param)
    res = run_bass_kernel_spmd(nc, in_maps, list(range(NCORES)))
    G = np.empty((B, C, H, W), dtype=np.float32)
    L = np.empty((B, C, H, W), dtype=np.float32)
    for i in range(NCORES):
        out = res.results[i]["out_all"]
        for bb in range(BPC):
            G[i * BPC + bb] = out[2 * bb].reshape(C, H, W)
            L[i * BPC + bb] = out[2 * bb + 1].reshape(C, H, W)
    return G, L
